# revision 29
# baseline (speedup 1.0000x reference)
"""Trainium2 Bass kernel for the ActorCriticCriterion (AIC) masked REINFORCE loss.

Reference computation (per the oracle):
    at_or_after_eos = cumsum(seq == 0, axis=1) > 0
    seq_z  = where(at_or_after_eos, 0, seq)
    mask   = concat([ones(B,1), (seq_z > 0)[:, :-1]], axis=1)
    loss   = sum(-logp * (reward - value) * mask) / sum(mask)

Identity used: mask[t] = AND(seq[0..t-1] != 0) with mask[0] = 1 — computed
directly with one DVE tensor_tensor_scan (op0=logical_and) per 128-row group,
writing to a shifted access pattern (the leading ones column is a memset).

Per [128, A, T] block:
    pool:  d = value - reward                       (gpsimd tensor_tensor)
    DVE:   mask via logical_and scan (shifted)
           q  = logp * d
           mq = q * mask
    PE:    ones[128,1].T @ mq  chunks -> PSUM num[1, A*T]  (accum over blocks)
           ones[128,1].T @ mask chunks -> PSUM den[1, A*T]
Outputs are the two [1, A*T] PSUM accumulators; the host sums them and
divides.  Sharding: pure data-parallel over B across 8 cores (1024 rows each).
"""

import os
import numpy as np

B, T = 8192, 1024
NCORES = 8
ROWS = B // NCORES          # rows per core
P = 128                     # SBUF partitions
A = 2                       # row-groups of 128 per block (tile = [128, A, T])
MMCHUNK = 512               # matmul free-dim chunk (one PSUM bank)

_CACHE: dict = {}


def _build_program(rows: int, d_engine: str = "pool"):
    """Build the Bass/Tile program for one core processing `rows` rows."""
    from contextlib import ExitStack

    import concourse.bacc as bacc
    import concourse.mybir as mybir
    import concourse.tile as tile

    nblk = rows // (P * A)
    assert nblk * P * A == rows

    f32 = mybir.dt.float32
    i32 = mybir.dt.int32
    Op = mybir.AluOpType

    # Bacc (not raw Bass): its compile pipeline splits multi-sem sync waits
    # into event-semaphore instructions — this walrus build allows at most
    # one wait per engine instruction.
    nc = bacc.Bacc()
    seq = nc.dram_tensor("seq", [rows, T], i32, kind="ExternalInput")
    lp = nc.dram_tensor("lp", [rows, T], f32, kind="ExternalInput")
    val = nc.dram_tensor("val", [rows, T], f32, kind="ExternalInput")
    rew = nc.dram_tensor("rew", [rows, T], f32, kind="ExternalInput")
    out_num = nc.dram_tensor("out_num", [1, A * T], f32, kind="ExternalOutput")
    out_den = nc.dram_tensor("out_den", [1, A * T], f32, kind="ExternalOutput")

    # Sub-blocks of (row_start, a_count): half-size first and last blocks
    # shorten the pipeline ramp and drain tail; full-size middle keeps 1MB
    # DMAs. Row coverage must tile [0, rows).
    subs = []
    r = 0
    layout = [1, 1] + [A] * ((rows // (P * A)) - 2) + [1, 1]
    for na in layout:
        subs.append((r, na))
        r += P * na
    assert r == rows

    # first/last sub-block writing each local-a row-group (for PSUM
    # accumulation start/stop flags)
    first_sb = {}
    last_sb = {}
    for si, (_, na) in enumerate(subs):
        for la in range(na):
            first_sb.setdefault(la, si)
            last_sb[la] = si

    def dram_sub(t, r0, na):
        # rows [r0, r0 + na*P) as [p, a, t] with row = r0 + a*P + p
        return t[r0:r0 + na * P, :].rearrange("(a p) t -> p a t", p=P)

    with ExitStack() as ctx:
        tc = ctx.enter_context(tile.TileContext(nc))
        const_pool = ctx.enter_context(tc.tile_pool(name="const", bufs=1))
        in_pool = ctx.enter_context(tc.tile_pool(name="in", bufs=3))
        scr_pool = ctx.enter_context(tc.tile_pool(name="scr", bufs=2))
        psum_pool = ctx.enter_context(
            tc.tile_pool(name="psum", bufs=1, space="PSUM"))

        ones = const_pool.tile([P, 1], f32)
        nc.vector.memset(ones[:], 1.0)

        num_ps = psum_pool.tile([1, A * T], f32)
        den_ps = psum_pool.tile([1, A * T], f32)

        for si, (r0, na) in enumerate(subs):
            seq_t = in_pool.tile([P, na, T], i32, tag="seq", bufs=4)
            lp_t = in_pool.tile([P, na, T], f32, tag="lp", bufs=4)
            val_t = in_pool.tile([P, na, T], f32, tag="val")
            rew_t = in_pool.tile([P, na, T], f32, tag="rew")
            # Two HWDGE rings in parallel: val/rew (feeding d) on the sync
            # ring, seq/lp (feeding scan and q) on the scalar ring.
            nc.sync.dma_start(out=val_t[:], in_=dram_sub(val, r0, na))
            nc.scalar.dma_start(out=seq_t[:], in_=dram_sub(seq, r0, na))
            nc.sync.dma_start(out=rew_t[:], in_=dram_sub(rew, r0, na))
            nc.scalar.dma_start(out=lp_t[:], in_=dram_sub(lp, r0, na))

            # d = value - reward. NOTE: gpsimd shares its SBUF port with the
            # vector engine — running this on pool slows concurrent DVE ops
            # ~2.9x, a net loss. Keep everything on DVE (pool idle).
            d = scr_pool.tile([P, na, T], f32, tag="d")
            eng = nc.gpsimd if d_engine == "pool" else nc.vector
            eng.tensor_tensor(out=d[:], in0=val_t[:], in1=rew_t[:],
                              op=Op.subtract)

            # mask[p,a,0] = 1; mask[p,a,t] = AND(seq[p,a,0..t-1] != 0)
            mask = scr_pool.tile([P, na, T], f32, tag="mask")
            nc.vector.memset(mask[:, :, 0:1], 1.0)
            for a in range(na):
                nc.vector.tensor_tensor_scan(
                    out=mask[:, a, 1:T], data0=seq_t[:, a, 0:T - 1],
                    data1=seq_t[:, a, 0:T - 1], initial=1.0,
                    op0=Op.logical_and, op1=Op.bypass)

            # den column sums can go to PE as soon as the mask exists.
            for a in range(na):
                for c in range(0, T, MMCHUNK):
                    sl = slice(a * T + c, a * T + c + MMCHUNK)
                    nc.tensor.matmul(
                        out=den_ps[:, sl], lhsT=ones[:],
                        rhs=mask[:, a, c:c + MMCHUNK],
                        start=(si == first_sb[a]), stop=(si == last_sb[a]))

            # q = logp * d ; mq = q * mask (mq reuses the dead d slot tag)
            q = scr_pool.tile([P, na, T], f32, tag="q")
            nc.vector.tensor_tensor(out=q[:], in0=lp_t[:], in1=d[:], op=Op.mult)
            mq = scr_pool.tile([P, na, T], f32, tag="d")
            nc.vector.tensor_tensor(out=mq[:], in0=q[:], in1=mask[:],
                                    op=Op.mult)

            for a in range(na):
                for c in range(0, T, MMCHUNK):
                    sl = slice(a * T + c, a * T + c + MMCHUNK)
                    nc.tensor.matmul(
                        out=num_ps[:, sl], lhsT=ones[:],
                        rhs=mq[:, a, c:c + MMCHUNK],
                        start=(si == first_sb[a]), stop=(si == last_sb[a]))

        # PSUM can't be DMA'd directly — bounce through SBUF on the (idle)
        # scalar engine, which sits closest to PSUM.
        num_sb = const_pool.tile([1, A * T], f32)
        den_sb = const_pool.tile([1, A * T], f32)
        nc.scalar.copy(num_sb[:], num_ps[:])
        nc.scalar.copy(den_sb[:], den_ps[:])
        nc.sync.dma_start(out=out_num[:], in_=num_sb[:])
        nc.sync.dma_start(out=out_den[:], in_=den_sb[:])

    nc.finalize()
    return nc


def kernel(sample_seq, sample_seqLogprobs, sample_value, sample_reward):
    from concourse.bass_utils import run_bass_kernel_spmd

    seq = np.ascontiguousarray(np.asarray(sample_seq, dtype=np.int32))
    lp = np.ascontiguousarray(np.asarray(sample_seqLogprobs, dtype=np.float32))
    val = np.ascontiguousarray(np.asarray(sample_value, dtype=np.float32))
    rew = np.ascontiguousarray(np.asarray(sample_reward, dtype=np.float32))
    assert seq.shape == (B, T)

    if "nc" not in _CACHE:
        _CACHE["nc"] = _build_program(
            ROWS, d_engine=os.environ.get("K_D_ENGINE", "dve"))
    nc = _CACHE["nc"]

    in_maps = []
    for c in range(NCORES):
        sl = slice(c * ROWS, (c + 1) * ROWS)
        in_maps.append({
            "seq": seq[sl], "lp": lp[sl], "val": val[sl], "rew": rew[sl],
        })

    trace = bool(int(os.environ.get("K_TRACE", "0")))
    res = run_bass_kernel_spmd(nc, in_maps, core_ids=list(range(NCORES)),
                               trace=trace)
    if trace:
        _CACHE["exec_time_ns"] = res.exec_time_ns
        _CACHE["trace"] = res.instructions_and_trace
    num = 0.0
    den = 0.0
    for r in res.results:
        num += float(np.asarray(r["out_num"], dtype=np.float64).sum())
        den += float(np.asarray(r["out_den"], dtype=np.float64).sum())
    return np.float32(num / den)


# revision 35
# speedup vs baseline: 1.0653x; 1.0653x over previous
"""Trainium2 Bass kernel for the ActorCriticCriterion (AIC) masked REINFORCE loss.

Reference computation (per the oracle):
    at_or_after_eos = cumsum(seq == 0, axis=1) > 0
    seq_z  = where(at_or_after_eos, 0, seq)
    mask   = concat([ones(B,1), (seq_z > 0)[:, :-1]], axis=1)
    loss   = sum(-logp * (reward - value) * mask) / sum(mask)

Identity used: mask[t] = AND(seq[0..t-1] != 0) with mask[0] = 1 — computed
directly with one DVE tensor_tensor_scan (op0=logical_and) per 128-row group,
writing to a shifted access pattern (the leading ones column is a memset).

Per [128, A, T] block:
    pool:  d = value - reward                       (gpsimd tensor_tensor)
    DVE:   mask via logical_and scan (shifted)
           q  = logp * d
           mq = q * mask
    PE:    ones[128,1].T @ mq  chunks -> PSUM num[1, A*T]  (accum over blocks)
           ones[128,1].T @ mask chunks -> PSUM den[1, A*T]
Outputs are the two [1, A*T] PSUM accumulators; the host sums them and
divides.  Sharding: pure data-parallel over B across 8 cores (1024 rows each).
"""

import os
import numpy as np

B, T = 8192, 1024
NCORES = 8
ROWS = B // NCORES          # rows per core
P = 128                     # SBUF partitions
A = 2                       # row-groups of 128 per block (tile = [128, A, T])
MMCHUNK = 512               # matmul free-dim chunk (one PSUM bank)

_CACHE: dict = {}


def _build_program(rows: int, d_engine: str = "pool"):
    """Build the Bass/Tile program for one core processing `rows` rows."""
    from contextlib import ExitStack

    import concourse.bacc as bacc
    import concourse.mybir as mybir
    import concourse.tile as tile

    nblk = rows // (P * A)
    assert nblk * P * A == rows

    f32 = mybir.dt.float32
    i32 = mybir.dt.int32
    Op = mybir.AluOpType

    # Bacc (not raw Bass): its compile pipeline splits multi-sem sync waits
    # into event-semaphore instructions — this walrus build allows at most
    # one wait per engine instruction.
    nc = bacc.Bacc()
    seq = nc.dram_tensor("seq", [rows, T], i32, kind="ExternalInput")
    lp = nc.dram_tensor("lp", [rows, T], f32, kind="ExternalInput")
    val = nc.dram_tensor("val", [rows, T], f32, kind="ExternalInput")
    rew = nc.dram_tensor("rew", [rows, T], f32, kind="ExternalInput")
    out_num = nc.dram_tensor("out_num", [1, A * T], f32, kind="ExternalOutput")
    out_den = nc.dram_tensor("out_den", [1, A * T], f32, kind="ExternalOutput")

    # Uniform blocks of A row-groups (half-size first/last measured worse:
    # extra DMA issues and sub-block boundary stalls outweighed the shorter
    # ramp/tail).
    subs = []
    r = 0
    layout = [A] * (rows // (P * A))
    for na in layout:
        subs.append((r, na))
        r += P * na
    assert r == rows

    # first/last sub-block writing each local-a row-group (for PSUM
    # accumulation start/stop flags)
    first_sb = {}
    last_sb = {}
    for si, (_, na) in enumerate(subs):
        for la in range(na):
            first_sb.setdefault(la, si)
            last_sb[la] = si

    def dram_sub(t, r0, na):
        # rows [r0, r0 + na*P) as [p, a, t] with row = r0 + a*P + p
        return t[r0:r0 + na * P, :].rearrange("(a p) t -> p a t", p=P)

    light_tail = bool(int(os.environ.get("K_LIGHT_TAIL", "1")))

    with ExitStack() as ctx:
        tc = ctx.enter_context(tile.TileContext(nc))
        if light_tail:
            # Replace Tile's end-of-kernel epilogue (drain + two all-engine
            # EVSEM barriers + 64-sem clear, ~8-9us) with just the final
            # drain. Safe for re-execution: the Bass preamble dma_reset +
            # sem_clear runs at the START of every execution, so leaving
            # semaphores dirty at kernel end is fine.
            import types

            from concourse.vector_clock import ScopedClock

            def _light_drain_and_barrier(self, tick_clock, wait_clock):
                drain_inst = self.nc.sync.drain()
                wait_clock.add_sem_waits(
                    drain_inst.ins,
                    ScopedClock({None: tick_clock.global_clock}))
                popped = self.nc._tile_sem_poison_stack.pop()
                assert popped is self._sem_poison
                self.sems.free_all()

            tc._drain_and_barrier = types.MethodType(
                _light_drain_and_barrier, tc)
        const_pool = ctx.enter_context(tc.tile_pool(name="const", bufs=1))
        in_pool = ctx.enter_context(tc.tile_pool(name="in", bufs=3))
        scr_pool = ctx.enter_context(tc.tile_pool(name="scr", bufs=2))
        psum_pool = ctx.enter_context(
            tc.tile_pool(name="psum", bufs=1, space="PSUM"))

        ones = const_pool.tile([P, 1], f32)
        nc.vector.memset(ones[:], 1.0)

        num_ps = psum_pool.tile([1, A * T], f32)
        den_ps = psum_pool.tile([1, A * T], f32)

        for si, (r0, na) in enumerate(subs):
            seq_t = in_pool.tile([P, na, T], i32, tag="seq", bufs=3)
            lp_t = in_pool.tile([P, na, T], f32, tag="lp", bufs=3)
            val_t = in_pool.tile([P, na, T], f32, tag="val", bufs=2)
            rew_t = in_pool.tile([P, na, T], f32, tag="rew", bufs=2)
            # Two HWDGE rings in parallel: val/rew (feeding d) on the sync
            # ring, seq/lp (feeding scan and q) on the scalar ring.
            nc.sync.dma_start(out=val_t[:], in_=dram_sub(val, r0, na))
            nc.scalar.dma_start(out=seq_t[:], in_=dram_sub(seq, r0, na))
            nc.sync.dma_start(out=rew_t[:], in_=dram_sub(rew, r0, na))
            nc.scalar.dma_start(out=lp_t[:], in_=dram_sub(lp, r0, na))

            # d = value - reward. NOTE: gpsimd shares its SBUF port with the
            # vector engine — running this on pool slows concurrent DVE ops
            # ~2.9x, a net loss. Keep everything on DVE (pool idle).
            d = scr_pool.tile([P, na, T], f32, tag="d")
            eng = nc.gpsimd if d_engine == "pool" else nc.vector
            eng.tensor_tensor(out=d[:], in0=val_t[:], in1=rew_t[:],
                              op=Op.subtract)

            # mask[p,a,0] = 1; mask[p,a,t] = AND(seq[p,a,0..t-1] != 0)
            mask = scr_pool.tile([P, na, T], f32, tag="mask", bufs=3)
            nc.vector.memset(mask[:, :, 0:1], 1.0)
            for a in range(na):
                nc.vector.tensor_tensor_scan(
                    out=mask[:, a, 1:T], data0=seq_t[:, a, 0:T - 1],
                    data1=seq_t[:, a, 0:T - 1], initial=1.0,
                    op0=Op.logical_and, op1=Op.bypass)

            # den column sums can go to PE as soon as the mask exists.
            for a in range(na):
                for c in range(0, T, MMCHUNK):
                    sl = slice(a * T + c, a * T + c + MMCHUNK)
                    nc.tensor.matmul(
                        out=den_ps[:, sl], lhsT=ones[:],
                        rhs=mask[:, a, c:c + MMCHUNK],
                        start=(si == first_sb[a]), stop=(si == last_sb[a]))

            # q = logp * d ; mq = q * mask
            q = scr_pool.tile([P, na, T], f32, tag="q")
            nc.vector.tensor_tensor(out=q[:], in0=lp_t[:], in1=d[:], op=Op.mult)
            mq = scr_pool.tile([P, na, T], f32, tag="mq", bufs=3)
            nc.vector.tensor_tensor(out=mq[:], in0=q[:], in1=mask[:],
                                    op=Op.mult)

            for a in range(na):
                for c in range(0, T, MMCHUNK):
                    sl = slice(a * T + c, a * T + c + MMCHUNK)
                    nc.tensor.matmul(
                        out=num_ps[:, sl], lhsT=ones[:],
                        rhs=mq[:, a, c:c + MMCHUNK],
                        start=(si == first_sb[a]), stop=(si == last_sb[a]))

        # PSUM can't be DMA'd directly — bounce through SBUF, with the halves
        # split across the scalar and vector engines so the two copies of
        # each output run in parallel.
        H = (A * T) // 2
        num_sb = const_pool.tile([1, A * T], f32)
        den_sb = const_pool.tile([1, A * T], f32)
        nc.scalar.copy(den_sb[:, 0:H], den_ps[:, 0:H])
        nc.vector.tensor_copy(den_sb[:, H:], den_ps[:, H:])
        nc.sync.dma_start(out=out_den[:], in_=den_sb[:])
        nc.scalar.copy(num_sb[:, 0:H], num_ps[:, 0:H])
        nc.vector.tensor_copy(num_sb[:, H:], num_ps[:, H:])
        nc.sync.dma_start(out=out_num[:], in_=num_sb[:])

    nc.finalize()
    return nc


def kernel(sample_seq, sample_seqLogprobs, sample_value, sample_reward):
    from concourse.bass_utils import run_bass_kernel_spmd

    seq = np.ascontiguousarray(np.asarray(sample_seq, dtype=np.int32))
    lp = np.ascontiguousarray(np.asarray(sample_seqLogprobs, dtype=np.float32))
    val = np.ascontiguousarray(np.asarray(sample_value, dtype=np.float32))
    rew = np.ascontiguousarray(np.asarray(sample_reward, dtype=np.float32))
    assert seq.shape == (B, T)

    if "nc" not in _CACHE:
        _CACHE["nc"] = _build_program(
            ROWS, d_engine=os.environ.get("K_D_ENGINE", "dve"))
    nc = _CACHE["nc"]

    in_maps = []
    for c in range(NCORES):
        sl = slice(c * ROWS, (c + 1) * ROWS)
        in_maps.append({
            "seq": seq[sl], "lp": lp[sl], "val": val[sl], "rew": rew[sl],
        })

    trace = bool(int(os.environ.get("K_TRACE", "0")))
    res = run_bass_kernel_spmd(nc, in_maps, core_ids=list(range(NCORES)),
                               trace=trace)
    if trace:
        _CACHE["exec_time_ns"] = res.exec_time_ns
        _CACHE["trace"] = res.instructions_and_trace
    num = 0.0
    den = 0.0
    for r in res.results:
        num += float(np.asarray(r["out_num"], dtype=np.float64).sum())
        den += float(np.asarray(r["out_den"], dtype=np.float64).sum())
    return np.float32(num / den)
